# revision 4
# baseline (speedup 1.0000x reference)
"""Trainium2 Bass kernel for dynamic-conv1d attention-scale module.

Computes out = x + x * scale where
  scale[b,c,h,w] = sum_k attn[b,k,h,w] * w_sum[k,c]
  attn = softmax_k(logits/T),  logits[b,k,h,w] = fc2 @ relu(fc1 * qm)
  w_sum = weight.sum(axis=1)

Device strategy (8 NeuronCores, data-parallel over batch x H-halves):
  * quality_map >= 0, fc1 is a 1x1 conv with no bias =>
    relu(fc1_w * q) == q * relu(fc1_w), so logits[k] = g[k]*q + b2[k]
    with g = fc2_w @ relu(fc1_w) (host-side weight-only folding).
  * softmax rows sum to 1 => 1 + scale = sum_k attn_k * (w_sum[k,c] + 1),
    so a single tiny PE matmul per tile produces (1+scale) in PSUM and one
    vector multiply forms the output tile.
Each core streams its 18.9 MB x-shard in [128 x 2048] fp32 tiles (1 MiB
DMAs), which keeps the kernel at the HBM roofline (~38 MB of traffic/core).
"""

import sys

if "/opt/trn_rl_repo" not in sys.path:
    sys.path.insert(0, "/opt/trn_rl_repo")

import numpy as np

import concourse.bacc as bacc
import concourse.mybir as mybir
from concourse.bass_utils import run_bass_kernel_spmd
from concourse.tile import TileContext

_B, _C, _H, _W = 4, 256, 192, 192
_K = 4
_TEMP = 34.0
_NCORES = 8
_HS = _H // 2            # 96 rows of H per shard
_N = _HS * _W            # 18432 pixels per core
_P = 128                 # SBUF partitions
_F = _N // _P            # 144
_CH = 2048               # pixels per main-loop tile (8 KB/partition)
_NT = _N // _CH          # 9 chunks
_MM = 512                # matmul moving free dim (one PSUM bank, fp32)
_DT = mybir.dt.float32


def _build_nc():
    nc = bacc.Bacc()
    x_d = nc.dram_tensor("x", [_C, _N], _DT, kind="ExternalInput")
    qm_d = nc.dram_tensor("qm", [_P, _F], _DT, kind="ExternalInput")
    w_d = nc.dram_tensor("w", [_K, _C], _DT, kind="ExternalInput")
    g_d = nc.dram_tensor("g", [_P, 2 * _K], _DT, kind="ExternalInput")
    y_d = nc.dram_tensor("y", [_C, _N], _DT, kind="ExternalOutput")

    with TileContext(nc) as tc:
        with (
            tc.tile_pool(name="const", bufs=1) as cpool,
            tc.tile_pool(name="attn", bufs=1) as apool,
            tc.tile_pool(name="xin", bufs=4) as xpool,
            tc.tile_pool(name="yout", bufs=4) as ypool,
            tc.tile_pool(name="ps", bufs=2, space="PSUM") as pspool,
        ):
            # Replicated constants: lhsT (w_sum+1) and per-partition scale/bias
            wt = cpool.tile([_K, _C], _DT)
            nc.sync.dma_start(out=wt[:, :], in_=w_d[:, :])
            gt = cpool.tile([_P, 2 * _K], _DT)
            nc.sync.dma_start(out=gt[:, :], in_=g_d[:, :])

            # ---- attention: all pointwise work in [128, 144] layout ----
            q = apool.tile([_P, _F], _DT)
            nc.sync.dma_start(out=q[:, :], in_=qm_d[:, :])
            e = apool.tile([_P, _K * _F], _DT)
            for k in range(_K):
                # e_k = exp((g_k/T) * q + b_k/T)
                nc.scalar.activation(
                    out=e[:, k * _F : (k + 1) * _F],
                    in_=q[:, :],
                    func=mybir.ActivationFunctionType.Exp,
                    bias=gt[:, _K + k : _K + k + 1],
                    scale=gt[:, k : k + 1],
                )
            d0 = apool.tile([_P, _F], _DT)
            d1 = apool.tile([_P, _F], _DT)
            nc.vector.tensor_add(out=d0[:, :], in0=e[:, 0:_F], in1=e[:, _F : 2 * _F])
            nc.vector.tensor_add(
                out=d1[:, :], in0=e[:, 2 * _F : 3 * _F], in1=e[:, 3 * _F : 4 * _F]
            )
            nc.vector.tensor_add(out=d0[:, :], in0=d0[:, :], in1=d1[:, :])
            r = apool.tile([_P, _F], _DT)
            nc.vector.reciprocal(out=r[:, :], in_=d0[:, :])
            a = apool.tile([_P, _K * _F], _DT)
            for k in range(_K):
                nc.vector.tensor_mul(
                    out=a[:, k * _F : (k + 1) * _F],
                    in0=e[:, k * _F : (k + 1) * _F],
                    in1=r[:, :],
                )
            # Row-ify: [128,144] partition-major -> [1,18432] pixel-major rows
            rows = apool.tile([_K, _N], _DT)
            for k in range(_K):
                nc.sync.dma_start(
                    out=rows[k : k + 1, :], in_=a[:, k * _F : (k + 1) * _F]
                )

            # ---- main stream: out = x * (1 + scale) ----
            for ch in range(_C // _P):
                lhsT = wt[:, ch * _P : (ch + 1) * _P]
                for t in range(_NT):
                    xt = xpool.tile([_P, _CH], _DT)
                    nc.sync.dma_start(
                        out=xt[:, :],
                        in_=x_d[ch * _P : (ch + 1) * _P, t * _CH : (t + 1) * _CH],
                    )
                    ps = pspool.tile([_P, _CH], _DT)
                    for j in range(_CH // _MM):
                        nc.tensor.matmul(
                            ps[:, j * _MM : (j + 1) * _MM],
                            lhsT,
                            rows[:, t * _CH + j * _MM : t * _CH + (j + 1) * _MM],
                            start=True,
                            stop=True,
                        )
                    ot = ypool.tile([_P, _CH], _DT)
                    nc.vector.tensor_mul(out=ot[:, :], in0=xt[:, :], in1=ps[:, :])
                    nc.scalar.dma_start(
                        out=y_d[ch * _P : (ch + 1) * _P, t * _CH : (t + 1) * _CH],
                        in_=ot[:, :],
                    )
    nc.compile()
    return nc


def _prepare_in_maps(x, quality_map, fc1_w, fc2_w, fc2_b, weight):
    x = np.asarray(x, dtype=np.float32)
    qm = np.asarray(quality_map, dtype=np.float32)
    fc1 = np.asarray(fc1_w, dtype=np.float32)
    fc2 = np.asarray(fc2_w, dtype=np.float32)
    b2 = np.asarray(fc2_b, dtype=np.float32)
    w = np.asarray(weight, dtype=np.float32)

    # Weight-only folding (host): g = fc2 @ relu(fc1); lhsT = w_sum + 1
    g = (fc2 @ np.maximum(fc1[:, 0], 0.0)).astype(np.float32)     # [K]
    w1 = (w.sum(axis=1) + 1.0).astype(np.float32)                 # [K, C]
    gb = np.concatenate([g / _TEMP, b2 / _TEMP]).astype(np.float32)  # [2K]
    gb_rep = np.ascontiguousarray(np.broadcast_to(gb, (_P, 2 * _K)))

    in_maps = []
    for core in range(_NCORES):
        b, half = divmod(core, 2)
        h0 = half * _HS
        xs = np.ascontiguousarray(x[b, :, h0 : h0 + _HS, :]).reshape(_C, _N)
        qs = np.ascontiguousarray(qm[b, 0, h0 : h0 + _HS, :]).reshape(_P, _F)
        in_maps.append({"x": xs, "qm": qs, "w": w1, "g": gb_rep})
    return in_maps


def _run(in_maps, **kwargs):
    nc = _build_nc()
    return run_bass_kernel_spmd(nc, in_maps, core_ids=list(range(_NCORES)), **kwargs)


def kernel(x, quality_map, fc1_w, fc2_w, fc2_b, weight):
    in_maps = _prepare_in_maps(x, quality_map, fc1_w, fc2_w, fc2_b, weight)
    res = _run(in_maps)
    out = np.empty((_B, _C, _H, _W), dtype=np.float32)
    for core in range(_NCORES):
        b, half = divmod(core, 2)
        h0 = half * _HS
        out[b, :, h0 : h0 + _HS, :] = res.results[core]["y"].reshape(_C, _HS, _W)
    return out


# revision 14
# speedup vs baseline: 1.1517x; 1.1517x over previous
"""Trainium2 Bass kernel for dynamic-conv1d attention-scale module.

Computes out = x + x * scale where
  scale[b,c,h,w] = sum_k attn[b,k,h,w] * w_sum[k,c]
  attn = softmax_k(logits/T),  logits[b,k,h,w] = fc2 @ relu(fc1 * qm)
  w_sum = weight.sum(axis=1)

Device strategy (8 NeuronCores, data-parallel over batch x H-halves):
  * quality_map >= 0, fc1 is a 1x1 conv with no bias =>
    relu(fc1_w * q) == q * relu(fc1_w), so logits[k] = g[k]*q + b2[k]
    with g = fc2_w @ relu(fc1_w) (host-side weight-only folding).
  * softmax rows sum to 1 => 1 + scale = sum_k attn_k * (w_sum[k,c] + 1),
    so a single tiny PE matmul per tile produces (1+scale) in PSUM and one
    vector multiply forms the output tile.
Each core streams its 18.9 MB x-shard in [128 x 2048] fp32 tiles (1 MiB
DMAs), which keeps the kernel at the HBM roofline (~38 MB of traffic/core).
"""

import sys

if "/opt/trn_rl_repo" not in sys.path:
    sys.path.insert(0, "/opt/trn_rl_repo")

import numpy as np

import concourse.bacc as bacc
import concourse.mybir as mybir
from concourse.bass_utils import run_bass_kernel_spmd
from concourse.tile import TileContext

_B, _C, _H, _W = 4, 256, 192, 192
_K = 4
_TEMP = 34.0
_NCORES = 8
_HS = _H // 2            # 96 rows of H per shard
_N = _HS * _W            # 18432 pixels per core
_P = 128                 # SBUF partitions
_AP = 64                 # partitions used for attention pointwise math
_AF = _N // _AP          # 288 pixels per partition there (1152B DMA runs)
_CH = 2048               # pixels per main-loop tile (8 KB/partition)
_NT = _N // _CH          # 9 chunks
_MM = 512                # matmul moving free dim (one PSUM bank, fp32)
_DT = mybir.dt.float32
_DTR = mybir.dt.float32r


def _build_nc():
    nc = bacc.Bacc()
    x_d = nc.dram_tensor("x", [_C, _N], _DT, kind="ExternalInput")
    qm_d = nc.dram_tensor("qm", [_AP, _AF], _DT, kind="ExternalInput")
    w_d = nc.dram_tensor("w", [_K, _C], _DTR, kind="ExternalInput")
    g_d = nc.dram_tensor("g", [_AP, 2 * _K], _DT, kind="ExternalInput")
    y_d = nc.dram_tensor("y", [_C, _N], _DT, kind="ExternalOutput")

    with TileContext(nc) as tc:
        with (
            tc.tile_pool(name="const", bufs=1) as cpool,
            tc.tile_pool(name="attn", bufs=1) as apool,
            tc.tile_pool(name="xin", bufs=4) as xpool,
            tc.tile_pool(name="yout", bufs=4) as ypool,
            tc.tile_pool(name="ps", bufs=2, space="PSUM") as pspool,
        ):
            # Replicated constants: lhsT (w_sum+1) and per-partition scale/bias.
            # All small DMAs ride the scalar (ACT) HWDGE queue, which is idle
            # early — the sync queue is reserved for the bulk x stream.
            wt = cpool.tile([_K, _C], _DTR)
            nc.scalar.dma_start(out=wt[:, :], in_=w_d[:, :])
            gt = cpool.tile([_AP, 2 * _K], _DT)
            nc.scalar.dma_start(out=gt[:, :], in_=g_d[:, :])

            # ---- attention: all pointwise work in [64, 288] layout ----
            q = apool.tile([_AP, _AF], _DT)
            nc.scalar.dma_start(out=q[:, :], in_=qm_d[:, :])
            e = apool.tile([_AP, _K * _AF], _DT)
            for k in range(_K):
                # e_k = exp((g_k/T) * q + b_k/T)
                nc.scalar.activation(
                    out=e[:, k * _AF : (k + 1) * _AF],
                    in_=q[:, :],
                    func=mybir.ActivationFunctionType.Exp,
                    bias=gt[:, _K + k : _K + k + 1],
                    scale=gt[:, k : k + 1],
                )
            d0 = apool.tile([_AP, _AF], _DT)
            d1 = apool.tile([_AP, _AF], _DT)
            nc.vector.tensor_add(out=d0[:, :], in0=e[:, 0:_AF], in1=e[:, _AF : 2 * _AF])
            nc.vector.tensor_add(
                out=d1[:, :], in0=e[:, 2 * _AF : 3 * _AF], in1=e[:, 3 * _AF : 4 * _AF]
            )
            nc.vector.tensor_add(out=d0[:, :], in0=d0[:, :], in1=d1[:, :])
            r = apool.tile([_AP, _AF], _DT)
            nc.vector.reciprocal(out=r[:, :], in_=d0[:, :])
            a = apool.tile([_AP, _K * _AF], _DTR)
            for k in range(_K):
                nc.vector.tensor_mul(
                    out=a[:, k * _AF : (k + 1) * _AF],
                    in0=e[:, k * _AF : (k + 1) * _AF],
                    in1=r[:, :],
                )
            # Row-ify: [64,288] partition-major -> [1,18432] pixel-major rows
            rows = apool.tile([_K, _N], _DTR)
            for k in range(_K):
                nc.scalar.dma_start(
                    out=rows[k : k + 1, :], in_=a[:, k * _AF : (k + 1) * _AF]
                )

            # ---- main stream: out = x * (1 + scale) ----
            rows_r = rows
            for ch in range(_C // _P):
                lhsT = wt[:, ch * _P : (ch + 1) * _P]
                for t in range(_NT):
                    xt = xpool.tile([_P, _CH], _DT)
                    nc.sync.dma_start(
                        out=xt[:, :],
                        in_=x_d[ch * _P : (ch + 1) * _P, t * _CH : (t + 1) * _CH],
                    )
                    ps = pspool.tile([_P, _CH], _DT)
                    for j in range(_CH // _MM):
                        nc.tensor.matmul(
                            ps[:, j * _MM : (j + 1) * _MM],
                            lhsT,
                            rows_r[:, t * _CH + j * _MM : t * _CH + (j + 1) * _MM],
                            start=True,
                            stop=True,
                        )
                    ot = ypool.tile([_P, _CH], _DT)
                    nc.vector.tensor_mul(out=ot[:, :], in0=xt[:, :], in1=ps[:, :])
                    nc.scalar.dma_start(
                        out=y_d[ch * _P : (ch + 1) * _P, t * _CH : (t + 1) * _CH],
                        in_=ot[:, :],
                    )
    nc.compile()
    return nc


def _prepare_in_maps(x, quality_map, fc1_w, fc2_w, fc2_b, weight):
    x = np.asarray(x, dtype=np.float32)
    qm = np.asarray(quality_map, dtype=np.float32)
    fc1 = np.asarray(fc1_w, dtype=np.float32)
    fc2 = np.asarray(fc2_w, dtype=np.float32)
    b2 = np.asarray(fc2_b, dtype=np.float32)
    w = np.asarray(weight, dtype=np.float32)

    # Weight-only folding (host): g = fc2 @ relu(fc1); lhsT = w_sum + 1
    g = (fc2 @ np.maximum(fc1[:, 0], 0.0)).astype(np.float32)     # [K]
    w1 = (w.sum(axis=1) + 1.0).astype(np.float32)                 # [K, C]
    gb = np.concatenate([g / _TEMP, b2 / _TEMP]).astype(np.float32)  # [2K]
    gb_rep = np.ascontiguousarray(np.broadcast_to(gb, (_AP, 2 * _K)))

    in_maps = []
    for core in range(_NCORES):
        b, half = divmod(core, 2)
        h0 = half * _HS
        xs = np.ascontiguousarray(x[b, :, h0 : h0 + _HS, :]).reshape(_C, _N)
        qs = np.ascontiguousarray(qm[b, 0, h0 : h0 + _HS, :]).reshape(_AP, _AF)
        in_maps.append({"x": xs, "qm": qs, "w": w1, "g": gb_rep})
    return in_maps


def _run(in_maps, **kwargs):
    nc = _build_nc()
    return run_bass_kernel_spmd(nc, in_maps, core_ids=list(range(_NCORES)), **kwargs)


def kernel(x, quality_map, fc1_w, fc2_w, fc2_b, weight):
    in_maps = _prepare_in_maps(x, quality_map, fc1_w, fc2_w, fc2_b, weight)
    res = _run(in_maps)
    out = np.empty((_B, _C, _H, _W), dtype=np.float32)
    for core in range(_NCORES):
        b, half = divmod(core, 2)
        h0 = half * _HS
        out[b, :, h0 : h0 + _HS, :] = res.results[core]["y"].reshape(_C, _HS, _W)
    return out
